# revision 7
# baseline (speedup 1.0000x reference)
"""Counter-propagation network forward pass on 8 TRN2 NeuronCores.

Computation (see reference):
    d2[b, h]  = ||x_b - k_h||^2            x: [B, D_IN], kohonen k: [H, D_IN]
    win[b]    = argmin_h d2[b, h]
    out[b, :] = grossberg.T[win[b], :]     grossberg: [D_OUT, H]

argmin_h d2 == argmax_h (x.k_h - 0.5*||k_h||^2)  (the ||x||^2 term is
constant per row), so the kernel computes scores = x @ k.T + bias with
bias_h = -0.5*||k_h||^2, takes a per-row argmax, and gathers rows of
grossberg.T by the winning index.

Sharding: data-parallel over batch. Each of the 8 cores gets 2048 rows of
x (pre-transposed to [D_IN, 2048] for the matmul lhsT layout) and a full
replica of the (small) kohonen / grossberg weights.

Per-core kernel:
  - fp32 matmuls (exact-precision path; bf16 would misrank near-tie rows):
    16 row-blocks x 8 H-chunks x 4 K-chunks of [128,128]x[128,512].
  - tensor_tensor_reduce fuses the bias add (PSUM + bias -> SBUF scores)
    with a chained running row-max.
  - max_index finds the argmax position over the full 4096 scores.
  - dma_gather pulls the winning grossberg.T rows straight from DRAM.
"""

import numpy as np

B, D_IN, H, D_OUT = 16384, 512, 4096, 1024
NCORES = 8
BL = B // NCORES          # 2048 rows per core
P = 128                   # partitions
MB = BL // P              # 16 row blocks per core
NC_FREE = 512             # matmul free dim (1 fp32 PSUM bank)
NCH = H // NC_FREE        # 8 H chunks
KCH = D_IN // P           # 4 contraction chunks
NQ = 4                    # SWDGE queues for the gathers

_CACHE = {}


def build_bass(repeat=None, gather_queues=None):
    """Build the per-core Bass program. `repeat` (timing harness only) wraps
    the whole body in a hardware For_i loop so device time can be measured
    as delta-wall/delta-iterations through the (slow, transfer-dominated)
    PJRT tunnel."""
    import concourse.bacc as bacc
    import concourse.bass as bass
    import concourse.mybir as mybir
    import concourse.tile as tile

    f32 = mybir.dt.float32

    if gather_queues is None:
        gather_queues = 1 if repeat is not None else NQ
    nc = bacc.Bacc("TRN2", target_bir_lowering=False,
                   num_swdge_queues=gather_queues)

    xT = nc.dram_tensor("xT", [D_IN, BL], f32, kind="ExternalInput")
    kT = nc.dram_tensor("kT", [D_IN, H], f32, kind="ExternalInput")
    gT = nc.dram_tensor("gT", [H, D_OUT], f32, kind="ExternalInput")
    bias = nc.dram_tensor("bias", [1, H], f32, kind="ExternalInput")
    out = nc.dram_tensor("out", [BL, D_OUT], f32, kind="ExternalOutput")
    widx = nc.dram_tensor("widx", [BL], mybir.dt.int32, kind="ExternalOutput")
    # DRAM bounce for rewrapping winner indices into the dma_gather layout
    # (idx i lives at partition i%16, column i//16, replicated over 128
    # partitions).
    w16d = nc.dram_tensor("w16d", [BL], mybir.dt.int16, kind="Internal")

    with tile.TileContext(nc) as tc:
        with (
            tc.tile_pool(name="consts", bufs=1) as consts,
            tc.tile_pool(name="psum", bufs=8, space="PSUM") as psum,
            tc.tile_pool(name="scores", bufs=2) as scores,
            tc.tile_pool(name="accs", bufs=4) as accs,
            tc.tile_pool(name="gouts", bufs=3) as gouts,
        ):
            import contextlib
            loop_ctx = (tc.For_i(0, repeat, 1,
                                 hint_engines=(mybir.EngineType.PE,))
                        if repeat is not None else contextlib.nullcontext())
            with loop_ctx:
                _emit_body(nc, bass, mybir, consts, psum, scores, accs, gouts,
                           xT, kT, gT, bias, out, widx, w16d, gather_queues)

    nc.compile()
    return nc


def _emit_body(nc, bass, mybir, consts, psum, scores, accs, gouts,
               xT, kT, gT, bias, out, widx, w16d, gather_queues=NQ):
    f32 = mybir.dt.float32
    if True:
        if True:
            kt, xt = [], []
            for k in range(KCH):
                tk = consts.tile([P, H], f32, tag=f"kt{k}")
                nc.sync.dma_start(out=tk[:, :], in_=kT.ap()[k * P:(k + 1) * P, :])
                kt.append(tk)
                tx = consts.tile([P, BL], f32, tag=f"xt{k}")
                nc.sync.dma_start(out=tx[:, :], in_=xT.ap()[k * P:(k + 1) * P, :])
                xt.append(tx)

            bias_bc = consts.tile([P, H], f32, tag="bias_bc")
            bap = bias.ap()
            bias_rep = bass.AP(tensor=bap.tensor, offset=0, ap=[[0, P], [1, H]])
            nc.sync.dma_start(out=bias_bc[:, :], in_=bias_rep)

            maxidx = consts.tile([P, 8 * MB], mybir.dt.uint16, tag="maxidx")
            idx16 = consts.tile([P, 8 * MB], mybir.dt.int16, tag="idx16")

            w16b = w16d.ap()
            outb = out.ap()
            gap = gT.ap()

            for m in range(MB):
                sc = scores.tile([P, H], f32, tag="sc")
                for n in range(NCH):
                    ps = psum.tile([P, NC_FREE], f32, tag="ps")
                    for k in range(KCH):
                        nc.tensor.matmul(
                            ps[:, :],
                            xt[k][:, m * P:(m + 1) * P],
                            kt[k][:, n * NC_FREE:(n + 1) * NC_FREE],
                            start=(k == 0),
                            stop=(k == KCH - 1),
                        )
                    nc.vector.tensor_add(
                        sc[:, n * NC_FREE:(n + 1) * NC_FREE],
                        ps[:, :],
                        bias_bc[:, n * NC_FREE:(n + 1) * NC_FREE],
                    )
                acc = accs.tile([P, 1], f32, tag="acc")
                nc.vector.reduce_max(acc[:, :], sc[:, :],
                                     axis=mybir.AxisListType.X)

                # max_index wants 8 search values per row; broadcast the row
                # max with a zero-stride AP.
                am = acc[:, :]
                in_max = bass.AP(tensor=am.tensor, offset=am.offset,
                                 ap=[list(am.ap[0]), [0, 8]])
                nc.vector.max_index(maxidx[:, 8 * m:8 * m + 8], in_max, sc[:, :])

                # Rewrap this block's 128 winner indices into the gather
                # layout via a DRAM bounce: col 0 of the top-8 -> w16d[m*128+p]
                nc.sync.dma_start(
                    out=bass.AP(tensor=w16b.tensor, offset=m * P, ap=[[1, P], [1, 1]]),
                    in_=maxidx[:, 8 * m:8 * m + 1].bitcast(mybir.dt.int16),
                )
                # w16d[m*128 + 16c + t] -> idx16[rep*16 + t, 8m + c], one DMA
                # per 16-partition replica stripe (DMA APs cap at 3 dims).
                for s in range(8):
                    nc.sync.dma_start(
                        out=idx16[16 * s:16 * (s + 1), 8 * m:8 * m + 8],
                        in_=bass.AP(tensor=w16b.tensor, offset=m * P,
                                    ap=[[1, 16], [16, 8]]),
                    )

                go = gouts.tile([P, 1, D_OUT], f32, tag="go")
                nc.gpsimd.dma_gather(
                    go[:, :, :],
                    gap,
                    idx16[:, 8 * m:8 * m + 8],
                    P,            # num_idxs
                    P,            # num_idxs_reg
                    D_OUT,        # elem_size
                    queue_num=m % gather_queues,
                )
                nc.sync.dma_start(
                    out=bass.AP(tensor=outb.tensor, offset=m * P * D_OUT,
                                ap=[[D_OUT, P], [1, D_OUT]]),
                    in_=go[:, 0, :],
                )

            # winner indices out: widen to int32, strided col-0 extract
            w32 = consts.tile([P, MB], mybir.dt.int32, tag="w32")
            mi = maxidx[:, :]
            nc.vector.tensor_copy(
                out=w32[:, :],
                in_=bass.AP(tensor=mi.tensor, offset=mi.offset,
                            ap=[list(mi.ap[0]), [8, MB]]),
            )
            wb = widx.ap()
            nc.sync.dma_start(
                out=bass.AP(tensor=wb.tensor, offset=0, ap=[[1, P], [P, MB]]),
                in_=w32[:, :],
            )


def prep_inputs(x, kohonen_weights, grossberg_weights):
    """Host-side shard/layout prep. Returns per-core input maps."""
    xTf = np.ascontiguousarray(x.T)                          # [D_IN, B]
    kTf = np.ascontiguousarray(kohonen_weights.T)            # [D_IN, H]
    gTf = np.ascontiguousarray(grossberg_weights.T)          # [H, D_OUT]
    bias = (-0.5 * (kohonen_weights.astype(np.float64) ** 2).sum(axis=1)
            ).astype(np.float32).reshape(1, H)
    in_maps = []
    for c in range(NCORES):
        in_maps.append({
            "xT": np.ascontiguousarray(xTf[:, c * BL:(c + 1) * BL]),
            "kT": kTf,
            "gT": gTf,
            "bias": bias,
        })
    return in_maps


def kernel(x, kohonen_weights, grossberg_weights):
    from concourse.bass_utils import run_bass_kernel_spmd

    if "nc" not in _CACHE:
        _CACHE["nc"] = build_bass()
    nc = _CACHE["nc"]

    in_maps = prep_inputs(x, kohonen_weights, grossberg_weights)
    res = run_bass_kernel_spmd(nc, in_maps, core_ids=list(range(NCORES)))
    output = np.concatenate([r["out"] for r in res.results], axis=0)
    winners = np.concatenate(
        [r["widx"].reshape(BL).astype(np.int32) for r in res.results], axis=0)
    return output, winners
